# revision 35
# baseline (speedup 1.0000x reference)
"""Trainium2 Bass kernel for the GCN autoencoder problem.

kernel(**inputs) takes the FULL unsharded inputs (x, edge_index, W, b, gamma,
beta), distributes across 8 NeuronCores internally, and returns the full
[12000, 12000] float32 output of:
  GCNConv (self-loops, symmetric norm) -> BatchNorm1d -> ReLU -> z @ z.T

Strategy (v2, dense-adjacency): the scatter-add is reformulated as a dense
matmul hT = xw'.T @ M where M[s, d] is the edge-multiplicity matrix shipped
as fp8_e4m3 (small integers -> exact).  The symmetric normalization
dinv[s]*dinv[d] is folded into xw' (src side, per-partition scale) and a
post-matmul column scale (dst side).  Each core owns 1536 dst nodes; h is
AllGathered (fp16, pipelined in 3 chunks), BN stats computed on device, and
the z @ z.T decode runs a block-tournament so only ~57% of the symmetric
output is computed and written (fp16); the host mirrors the rest.

Self-contained: only needs numpy + ml_dtypes + the concourse (Bass) runtime.
"""

import numpy as np
import ml_dtypes

import concourse.bass as bass
import concourse.bacc as bacc
import concourse.mybir as mybir
import concourse.tile as tile

N = 12000
C_IN = 256
F = 128
P = 128
NCORES = 8
NW = 12                      # 128-blocks per core (rows)
NLOC = NW * P                # 1536 nodes per shard (padded)
NPAD = NCORES * NLOC         # 12288
NBLK = NPAD // P             # 96 blocks of 128
TPC = 8                      # blocks per residue class (96/12)
NQ = 3                       # dst chunks per core
DQ = 512
SBATCH = 12                  # src-blocks per A DMA batch
BN_EPS = 1e-5
AT_COLS = NQ * NBLK * DQ     # 147456

# residue-class tournament: C_k = classes whose columns block-row-slot k computes
CLS = []
for k in range(NW):
    cs = [k] + [(k + d) % NW for d in range(1, 6)]
    if k < 6:
        cs.append((k + 6) % NW)
    CLS.append(sorted(cs))
SLOT_W = [len(c) * TPC * P for c in CLS]          # output cols per slot
SLOT_OFF = np.concatenate([[0], np.cumsum(SLOT_W)]).astype(int)
OUT_W = int(SLOT_OFF[-1])                          # 79872

AF = mybir.ActivationFunctionType
ALU = mybir.AluOpType


# --------------------------------------------------------------------------
# Host-side preprocessing: indices -> dense fp8 multiplicity matrix + scales.
# --------------------------------------------------------------------------

def preprocess(x, edge_index, W, gamma, beta):
    src = np.asarray(edge_index[0]).astype(np.int64)
    dst = np.asarray(edge_index[1]).astype(np.int64)
    src_all = np.concatenate([src, np.arange(N, dtype=np.int64)])
    dst_all = np.concatenate([dst, np.arange(N, dtype=np.int64)])
    deg = np.bincount(dst_all, minlength=N).astype(np.float32)
    dinv_pad = np.ones(NPAD, dtype=np.float32)
    dinv_pad[:N] = 1.0 / np.sqrt(deg)

    counts = np.zeros((NPAD, NPAD), dtype=np.uint8)
    np.add.at(counts, (src_all, dst_all), 1)
    M8 = counts.astype(ml_dtypes.float8_e4m3)     # exact small ints

    # fold the src-side dinv into x rows (commutes with @W): xw' = (dinv*x)@W
    xpad = np.zeros((NPAD, C_IN), dtype=np.float16)
    xpad[:N] = (np.asarray(x, dtype=np.float32)
                * dinv_pad[:N, None]).astype(np.float16)
    # pack x^T so each 12-block group is one contiguous per-partition DMA:
    # xT[p, (g*2+half)*1536 + col] = x[g*1536+col, half*128+p]
    xT = np.ascontiguousarray(
        xpad.T.reshape(2, P, 8, NLOC).transpose(1, 2, 0, 3)
        .reshape(P, 2 * NPAD))
    W16 = np.asarray(W, dtype=np.float32).astype(np.float16)  # [256, 128]
    gamma2 = np.asarray(gamma, dtype=np.float32).reshape(F, 1)
    beta2 = np.asarray(beta, dtype=np.float32).reshape(F, 1)
    dinv_sb = np.ascontiguousarray(dinv_pad.reshape(NBLK, P).T)  # [128, 96]

    in_maps = []
    for c in range(NCORES):
        Ml = M8[:, c * NLOC:(c + 1) * NLOC]
        A_packed = np.ascontiguousarray(
            Ml.reshape(NBLK, P, NQ, DQ).transpose(1, 2, 0, 3)
              .reshape(P, AT_COLS))
        dinvd = np.ascontiguousarray(np.broadcast_to(
            dinv_pad[c * NLOC:(c + 1) * NLOC], (P, NLOC)).astype(np.float32))
        in_maps.append({
            "xT": xT,
            "Wt": W16,
            "gamma": gamma2,
            "beta": beta2,
            "dinv_sb": dinv_sb,
            "A_packed": A_packed,
            "dinvd": dinvd,
        })
    return in_maps, None


# --------------------------------------------------------------------------
# Device program (one SPMD program for all 8 cores).
# --------------------------------------------------------------------------

def build(meta=None, bench_phase=None, bench_r=8, repeat=1, upto=4):
    nc = bacc.Bacc("TRN2", target_bir_lowering=False, debug=False,
                   num_devices=NCORES)
    f32 = mybir.dt.float32
    fp16 = mybir.dt.float16
    fp8 = mybir.dt.float8e4

    xT_d = nc.dram_tensor("xT", [P, 2 * NPAD], fp16, kind="ExternalInput")
    Wt_d = nc.dram_tensor("Wt", [C_IN, F], fp16, kind="ExternalInput")
    gamma_d = nc.dram_tensor("gamma", [F, 1], f32, kind="ExternalInput")
    beta_d = nc.dram_tensor("beta", [F, 1], f32, kind="ExternalInput")
    dinv_d = nc.dram_tensor("dinv_sb", [P, NBLK], f32, kind="ExternalInput")
    A_d = nc.dram_tensor("A_packed", [P, AT_COLS], fp8, kind="ExternalInput")
    dinvd_d = nc.dram_tensor("dinvd", [P, NLOC], f32, kind="ExternalInput")
    out_d = nc.dram_tensor("out", [P, OUT_W], fp16, kind="ExternalOutput")

    rg = [list(range(NCORES))]

    with tile.TileContext(nc) as tc:
      for rep in range(repeat):
        with tc.tile_pool(name="const", bufs=1) as const, \
             tc.tile_pool(name="big", bufs=1) as big, \
             tc.tile_pool(name="px", bufs=2) as px, \
             tc.tile_pool(name="pxps", bufs=2, space="PSUM") as pxps, \
             tc.tile_pool(name="pa", bufs=4) as pa, \
             tc.tile_pool(name="pcps", bufs=2, space="PSUM") as pcps, \
             tc.tile_pool(name="pb", bufs=1) as pb, \
             tc.tile_pool(name="pd", bufs=2) as pd, \
             tc.tile_pool(name="pdps", bufs=4, space="PSUM") as pdps, \
             tc.tile_pool(name="dram", bufs=1, space="DRAM") as dram:
            # ---------------- constants ----------------
            W_sb = const.tile([P, 2 * F], fp16)
            nc.sync.dma_start(W_sb[:, 0:F], Wt_d[0:P, :])
            nc.sync.dma_start(W_sb[:, F:2 * F], Wt_d[P:2 * P, :])
            gamma_sb = const.tile([P, 1], f32)
            nc.sync.dma_start(gamma_sb[:], gamma_d[:, :])
            beta_sb = const.tile([P, 1], f32)
            nc.sync.dma_start(beta_sb[:], beta_d[:, :])
            dinvd_sb = const.tile([P, NLOC], f32)
            nc.sync.dma_start(dinvd_sb[:], dinvd_d[:, :])

            # persistent tiles
            xw_all = big.tile([P, NPAD], fp16)      # xw' blocks, [s-part, f]
            hT_loc = big.tile([P, NLOC], fp16)      # local h, feature-major
            hT_full = big.tile([P, NPAD], fp16)
            zT_loc = big.tile([P, NLOC], fp16)
            zT_full = big.tile([P, NPAD], fp16)
            sq_scr = big.tile([P, TPC * DQ], fp16)  # Square scratch
            ssum_p = big.tile([P, NQ], f32)
            ssq_p = big.tile([P, NQ], f32)
            ssum_p8 = big.tile([P, NCORES], f32)
            ssq_p8 = big.tile([P, NCORES], f32)

            ag_in = [dram.tile([P, DQ], fp16, name=f"agi{q}_{rep}")
                     for q in range(NQ)]
            ag_out = [dram.tile([NCORES * P, DQ], fp16, addr_space="Shared",
                                name=f"ago{q}_{rep}") for q in range(NQ)]
            agi1 = dram.tile([P, NLOC], fp16, name=f"agione_{rep}")
            ago1 = dram.tile([NCORES * P, NLOC], fp16, addr_space="Shared",
                             name=f"agoone_{rep}")

            hT_v = hT_full[:].rearrange("p (r q d) -> p r q d",
                                        r=NCORES, q=NQ)
            sq_v = sq_scr[:].rearrange("p (r d) -> p r d", r=NCORES)
            zv = zT_full[:].rearrange("p (t c) -> p t c", t=TPC)
            noag = bench_phase in ("noag", "fullnoag")

            # ------- phase X: xw' = (dinv[s]*x) @ W, all 96 blocks ----------
            def phasex():
                for g in range(8):
                    xs = px.tile([P, 2 * NLOC], fp16, tag="xs")
                    nc.sync.dma_start(
                        xs[:], xT_d[:, 2 * g * NLOC:2 * (g + 1) * NLOC])
                    for bq in range(NW // 4):       # 4 blocks per PSUM tile
                        xwps = pxps.tile([P, 4 * F], f32, tag="xwps")
                        for j in range(4):
                            b = bq * 4 + j
                            nc.tensor.matmul(xwps[:, j * F:(j + 1) * F],
                                             lhsT=xs[:, b * P:(b + 1) * P],
                                             rhs=W_sb[:, 0:F],
                                             start=True, stop=False)
                            nc.tensor.matmul(xwps[:, j * F:(j + 1) * F],
                                             lhsT=xs[:, NLOC + b * P:
                                                     NLOC + (b + 1) * P],
                                             rhs=W_sb[:, F:2 * F],
                                             start=False, stop=True)
                        s0 = (g * NW + bq * 4) * P
                        if bq % 2 == 0:
                            nc.scalar.copy(xw_all[:, s0:s0 + 4 * P], xwps[:])
                        else:
                            nc.vector.tensor_copy(xw_all[:, s0:s0 + 4 * P],
                                                  xwps[:])

            # ------- phase C: hT = xw'.T @ M, dst-chunked, AG pipelined -----
            def conv_chunk(q):
                hps = pcps.tile([P, DQ], f32, tag="hps")
                for batch in range(NBLK // SBATCH):
                    asb = pa.tile([P, SBATCH * DQ], fp8, tag="asb")
                    off = (q * NBLK + batch * SBATCH) * DQ
                    nc.sync.dma_start(asb[:],
                                      A_d[:, off:off + SBATCH * DQ])
                    for j in range(SBATCH):
                        s = batch * SBATCH + j
                        nc.tensor.matmul(hps[:],
                                         lhsT=xw_all[:, s * P:(s + 1) * P],
                                         rhs=asb[:, j * DQ:(j + 1) * DQ],
                                         start=(s == 0), stop=(s == NBLK - 1))
                nc.vector.tensor_tensor(
                    out=hT_loc[:, q * DQ:(q + 1) * DQ], in0=hps[:],
                    in1=dinvd_sb[:, q * DQ:(q + 1) * DQ], op=ALU.mult)

            def ag_chunk(q, agi=None, ago=None):
                # keep all AG-side DMAs on the ACT HWDGE ring so they never
                # head-of-line-block the A-stream/out DMAs on the SP ring
                agi = ag_in[q] if agi is None else agi
                ago = ag_out[q] if ago is None else ago
                nc.scalar.dma_start(agi[:, :],
                                    hT_loc[:, q * DQ:(q + 1) * DQ])
                if noag:
                    for r in range(NCORES):
                        nc.scalar.dma_start(
                            hT_full[:, r * NLOC + q * DQ:
                                    r * NLOC + (q + 1) * DQ],
                            agi[:, :])
                else:
                    nc.gpsimd.collective_compute(
                        "AllGather", ALU.bypass, replica_groups=rg,
                        ins=[agi.opt()], outs=[ago.opt()])
                    for r in range(NCORES):
                        nc.scalar.dma_start(
                            hT_full[:, r * NLOC + q * DQ:
                                    r * NLOC + (q + 1) * DQ],
                            ago[r * P:(r + 1) * P, :])
                nc.vector.reduce_sum(out=ssum_p[:, q:q + 1],
                                     in_=hT_v[:, :, q, :],
                                     axis=mybir.AxisListType.XY)
                nc.scalar.activation(sq_v[:], hT_v[:, :, q, :],
                                     AF.Square,
                                     accum_out=ssq_p[:, q:q + 1])

            def ag_stage(q):
                # stage this chunk of the AG input while conv continues
                nc.scalar.dma_start(agi1[:, q * DQ:(q + 1) * DQ],
                                    hT_loc[:, q * DQ:(q + 1) * DQ])

            hT_r = hT_full[:].rearrange("p (r d) -> p r d", r=NCORES)
            sqr_v = sq_scr[:].rearrange("p (a d) -> p a d", a=2)

            def ag_all():
                # one AllGather of the whole local hT (input staged per
                # chunk during conv); readback split across both HWDGE
                # rings with per-slice stats as slices land
                if noag:
                    for r in range(NCORES):
                        eng = nc.scalar if r % 2 else nc.sync
                        eng.dma_start(
                            hT_full[:, r * NLOC:(r + 1) * NLOC], agi1[:, :])
                else:
                    nc.gpsimd.collective_compute(
                        "AllGather", ALU.bypass, replica_groups=rg,
                        ins=[agi1.opt()], outs=[ago1.opt()])
                    for r in range(NCORES):
                        eng = nc.scalar if r % 2 else nc.sync
                        eng.dma_start(
                            hT_full[:, r * NLOC:(r + 1) * NLOC],
                            ago1[r * P:(r + 1) * P, :])
                for r in range(NCORES):
                    nc.vector.reduce_sum(out=ssum_p8[:, r:r + 1],
                                         in_=hT_r[:, r, :],
                                         axis=mybir.AxisListType.XY)
                    nc.scalar.activation(sqr_v[:, r % 2, 0:NLOC],
                                         hT_r[:, r, :], AF.Square,
                                         accum_out=ssq_p8[:, r:r + 1])

            # ---------------- phase B: batchnorm + relu ----------------------
            def phaseb(wide=False):
                ssum = pb.tile([P, 1], f32, tag="ssum")
                nc.vector.reduce_sum(out=ssum[:],
                                     in_=ssum_p8[:] if wide else ssum_p[:],
                                     axis=mybir.AxisListType.X)
                ssq = pb.tile([P, 1], f32, tag="ssq")
                nc.vector.reduce_sum(out=ssq[:],
                                     in_=ssq_p8[:] if wide else ssq_p[:],
                                     axis=mybir.AxisListType.X)
                mean = pb.tile([P, 1], f32, tag="mean")
                nc.vector.tensor_scalar_mul(mean[:], ssum[:], 1.0 / N)
                ex2 = pb.tile([P, 1], f32, tag="ex2")
                nc.vector.tensor_scalar_mul(ex2[:], ssq[:], 1.0 / N)
                m2 = pb.tile([P, 1], f32, tag="m2")
                nc.vector.tensor_mul(m2[:], mean[:], mean[:])
                var = pb.tile([P, 1], f32, tag="var")
                nc.vector.tensor_tensor(out=var[:], in0=ex2[:], in1=m2[:],
                                        op=ALU.subtract)
                eps_sb = pb.tile([P, 1], f32, tag="eps")
                nc.gpsimd.memset(eps_sb[:], BN_EPS)
                sd = pb.tile([P, 1], f32, tag="sd")
                nc.scalar.activation(sd[:], var[:], AF.Sqrt,
                                     bias=eps_sb[:, :1])
                rstd = pb.tile([P, 1], f32, tag="rstd")
                nc.vector.reciprocal(rstd[:], sd[:])
                scale_f = pb.tile([P, 1], f32, tag="scalef")
                nc.vector.tensor_mul(scale_f[:], rstd[:], gamma_sb[:])
                msc = pb.tile([P, 1], f32, tag="msc")
                nc.vector.tensor_mul(msc[:], mean[:], scale_f[:])
                shift_f = pb.tile([P, 1], f32, tag="shiftf")
                nc.vector.tensor_tensor(out=shift_f[:], in0=beta_sb[:],
                                        in1=msc[:], op=ALU.subtract)
                nc.scalar.activation(zT_loc[:], hT_loc[:], AF.Relu,
                                     bias=shift_f[:, :1], scale=scale_f[:, :1])
                # split by t-halves so decode's half-0 matmuls start earlier
                nc.scalar.activation(zT_full[:, 0:NPAD // 2],
                                     hT_full[:, 0:NPAD // 2], AF.Relu,
                                     bias=shift_f[:, :1], scale=scale_f[:, :1])
                nc.scalar.activation(zT_full[:, NPAD // 2:NPAD],
                                     hT_full[:, NPAD // 2:NPAD], AF.Relu,
                                     bias=shift_f[:, :1], scale=scale_f[:, :1])

            # ---------------- phase D: decode z @ z.T (upper classes) --------
            def phased():
                for k in range(NW):
                    ob = pd.tile([P, max(SLOT_W)], fp16, tag="ob")
                    for ci, m in enumerate(CLS[k]):
                        for half in range(2):
                            ops = pdps.tile([P, 512], f32, tag="ops")
                            rhs = zv[:, half * 4:(half + 1) * 4,
                                     m * P:(m + 1) * P]
                            nc.tensor.matmul(
                                ops[:],
                                lhsT=zT_loc[:, k * P:(k + 1) * P],
                                rhs=rhs, start=True, stop=True)
                            o0 = ci * TPC * P + half * 512
                            if (ci + half) % 2 == 0:
                                nc.vector.tensor_copy(
                                    ob[:, o0:o0 + 512], ops[:])
                            else:
                                nc.scalar.copy(ob[:, o0:o0 + 512], ops[:])
                    nc.sync.dma_start(
                        out_d[:, int(SLOT_OFF[k]):int(SLOT_OFF[k]) + SLOT_W[k]],
                        ob[:, :SLOT_W[k]])

            def merged_x_conv0():
                # interleave xw production (per 12-block group) with conv q=0
                # matmuls on the just-produced blocks; A q0 batch g aligns.
                hps = pcps.tile([P, DQ], f32, tag="hps")
                for g in range(8):
                    xs = px.tile([P, 2 * NLOC], fp16, tag="xs")
                    nc.sync.dma_start(
                        xs[:], xT_d[:, 2 * g * NLOC:2 * (g + 1) * NLOC])
                    for bq in range(NW // 4):
                        xwps = pxps.tile([P, 4 * F], f32, tag="xwps")
                        for j in range(4):
                            b = bq * 4 + j
                            nc.tensor.matmul(xwps[:, j * F:(j + 1) * F],
                                             lhsT=xs[:, b * P:(b + 1) * P],
                                             rhs=W_sb[:, 0:F],
                                             start=True, stop=False)
                            nc.tensor.matmul(xwps[:, j * F:(j + 1) * F],
                                             lhsT=xs[:, NLOC + b * P:
                                                     NLOC + (b + 1) * P],
                                             rhs=W_sb[:, F:2 * F],
                                             start=False, stop=True)
                        s0 = (g * NW + bq * 4) * P
                        if bq % 2 == 0:
                            nc.scalar.copy(xw_all[:, s0:s0 + 4 * P], xwps[:])
                        else:
                            nc.vector.tensor_copy(xw_all[:, s0:s0 + 4 * P],
                                                  xwps[:])
                    asb = pa.tile([P, SBATCH * DQ], fp8, tag="asb")
                    off = g * SBATCH * DQ
                    nc.sync.dma_start(asb[:], A_d[:, off:off + SBATCH * DQ])
                    for j in range(SBATCH):
                        s = g * SBATCH + j
                        nc.tensor.matmul(hps[:],
                                         lhsT=xw_all[:, s * P:(s + 1) * P],
                                         rhs=asb[:, j * DQ:(j + 1) * DQ],
                                         start=(s == 0), stop=(s == NBLK - 1))
                nc.vector.tensor_tensor(
                    out=hT_loc[:, 0:DQ], in0=hps[:],
                    in1=dinvd_sb[:, 0:DQ], op=ALU.mult)

            def whole():
                merged_x_conv0()
                ag_stage(0)
                for q in range(1, NQ):
                    conv_chunk(q)
                    ag_stage(q)
                ag_all()
                phaseb(wide=True)
                phased()

            if bench_phase == "xw":
                with tc.For_i(0, bench_r, 1):
                    phasex()
            elif bench_phase == "conv":
                phasex()
                with tc.For_i(0, bench_r, 1):
                    for q in range(NQ):
                        conv_chunk(q)
            elif bench_phase == "dec":
                whole_once = [phasex()]
                for q in range(NQ):
                    conv_chunk(q)
                    ag_chunk(q)
                phaseb()
                with tc.For_i(0, bench_r, 1):
                    phased()
            elif bench_phase == "ag":
                phasex()
                for q in range(NQ):
                    conv_chunk(q)
                    ag_chunk(q)
                for it in range(bench_r - 1):
                    for q in range(NQ):
                        agi = dram.tile([P, DQ], fp16,
                                        name=f"bagi{q}_{it}_{rep}")
                        ago = dram.tile([NCORES * P, DQ], fp16,
                                        addr_space="Shared",
                                        name=f"bago{q}_{it}_{rep}")
                        ag_chunk(q, agi, ago)
                phaseb()
                phased()
            elif bench_phase == "fullnoag":
                with tc.For_i(0, bench_r, 1):
                    whole()
            else:
                if upto == 1:
                    phasex()
                elif upto == 2:
                    phasex()
                    for q in range(NQ):
                        conv_chunk(q)
                        ag_chunk(q)
                elif upto == 3:
                    phasex()
                    for q in range(NQ):
                        conv_chunk(q)
                        ag_chunk(q)
                    phaseb()
                else:
                    whole()
    nc.compile()
    return nc


# --------------------------------------------------------------------------
# Host-side unsharding: unpack class-layout, mirror the missing triangle.
# --------------------------------------------------------------------------

def assemble_output(results):
    full = np.zeros((NPAD, NPAD), dtype=np.float32)
    filled = np.zeros((NBLK, NBLK), dtype=bool)
    for c in range(NCORES):
        o = results[c]["out"].astype(np.float32)  # [P, OUT_W] fp16 -> f32
        for k in range(NW):
            a = c * NW + k  # global row block
            slot = o[:, int(SLOT_OFF[k]):int(SLOT_OFF[k]) + SLOT_W[k]]
            for ci, m in enumerate(CLS[k]):
                for t in range(TPC):
                    b = t * NW + m  # global col block
                    full[a * P:(a + 1) * P, b * P:(b + 1) * P] = \
                        slot[:, ci * TPC * P + t * P:(ci * TPC + t + 1) * P]
                    filled[a, b] = True
    for a in range(NBLK):
        for b in range(NBLK):
            if not filled[a, b]:
                full[a * P:(a + 1) * P, b * P:(b + 1) * P] = \
                    full[b * P:(b + 1) * P, a * P:(a + 1) * P].T
    return np.ascontiguousarray(full[:N, :N])


from concourse import bass_utils

_CACHE = {}


def kernel(x, edge_index, W, b, gamma, beta):
    in_maps, meta = preprocess(x, edge_index, W, gamma, beta)
    if "nc" not in _CACHE:
        _CACHE["nc"] = build(meta)
    nc = _CACHE["nc"]
    res = bass_utils.run_bass_kernel_spmd(
        nc, in_maps, core_ids=list(range(NCORES)))
    return assemble_output(res.results)
